# revision 17
# baseline (speedup 1.0000x reference)
"""HOPELoRALayer kernel for 8 Trainium2 NeuronCores.

Math identity used (exact):
  gates = softmax(z, axis=-1) over 3 timescales, and the reference takes
  gate_scale = mean(gates, axis=-1) = 1/3 exactly (softmax rows sum to 1).
  So the whole gate network is a constant 1/3 and the LoRA branch folds
  into the base weight per batch:
    W_eff_b = base_w + (ALPHA/3) * pu_w @ diag(1 + mem_b) @ pd_w
    out[b]  = x[b] @ W_eff_b^T + base_b

Per-core work (batch b on core b): one [4096,1024] x [1024,1024] GEMM
+ bias.  fp32 data, fp32r (full-rate) matmuls, PE transposes for x^T.
"""

import numpy as np

import concourse.bass as bass
import concourse.bacc as bacc
import concourse.mybir as mybir
import concourse.tile as tile
from concourse.bass_utils import run_bass_kernel_spmd
from concourse.masks import make_identity

B, S, D = 8, 4096, 1024
P = 128
NT = S // P  # 32 token tiles per core
KC = D // P  # 8 contraction chunks
ALPHA = 1.0

_F32 = mybir.dt.float32
_F32R = mybir.dt.float32r

_NC_CACHE = {}
LAST_RESULTS = None  # stashed BassKernelResults for test harness introspection


def _build_nc():
    # Bacc (not raw Bass): its compile() pass moves excess matmul waits to
    # ldweights / event semaphores — cayman self-loading fp32r matmuls only
    # support a single sync wait.
    nc = bacc.Bacc(None)
    x_ext = nc.declare_dram_parameter("x", [S, D], _F32, isOutput=False)
    w_ext = nc.declare_dram_parameter("w_t", [D, D], _F32R, isOutput=False)
    bias_ext = nc.declare_dram_parameter("bias_bc", [P, D], _F32, isOutput=False)
    out_ext = nc.declare_dram_parameter("out", [S, D], _F32, isOutput=True)

    with tile.TileContext(nc) as tc:
        with (
            tc.tile_pool(name="const", bufs=1) as cpool,
            tc.tile_pool(name="wpool", bufs=1) as wpool,
            tc.tile_pool(name="xin", bufs=3) as xpool,
            tc.tile_pool(name="xt", bufs=3) as xtpool,
            tc.tile_pool(name="obuf", bufs=3) as opool,
            tc.tile_pool(name="pst", bufs=4, space="PSUM") as pst_pool,
            tc.tile_pool(name="psacc", bufs=2, space="PSUM") as acc_pool,
        ):
            # Transposes stay plain f32: the fp32r transpose path crashed the
            # exec unit on HW (fp32r is only reliable via self-loading
            # matmuls); the f32r rounding happens in the ACT copy to SBUF.
            ident = cpool.tile([P, P], _F32)
            make_identity(nc, ident[:])

            bias_sb = cpool.tile([P, D], _F32)
            nc.sync.dma_start(bias_sb[:], bias_ext[:])

            # Weights: 16 separate [128,512] tiles so the first matmul only
            # waits on a 256KB DMA, not the full 4MB weight load.
            w_sb = [[None, None] for _ in range(KC)]
            for k in range(KC):
                for h in range(2):
                    wk = wpool.tile([P, 512], _F32R, tag=f"w{k}_{h}")
                    nc.sync.dma_start(
                        wk[:], w_ext[k * P : (k + 1) * P, h * 512 : (h + 1) * 512]
                    )
                    w_sb[k][h] = wk

            for i in range(NT):
                x_in = xpool.tile([P, D], _F32)
                nc.sync.dma_start(x_in[:], x_ext[i * P : (i + 1) * P, :])

                # Transpose x tile: 8x [128t,128d] -> [128d,128t] via PE,
                # staged 4-at-a-time through one PSUM bank, ACT copies to
                # SBUF.  Two separate half-tiles so GEMM k<4 never waits on
                # the second copy.
                xT = []
                for half in range(2):
                    ps_t = pst_pool.tile([P, 512], _F32)
                    for j in range(4):
                        k = half * 4 + j
                        nc.tensor.transpose(
                            ps_t[:, j * P : (j + 1) * P],
                            x_in[:, k * P : (k + 1) * P],
                            ident[:],
                        )
                    xT_h = xtpool.tile([P, 512], _F32R, tag=f"xt{half}")
                    nc.scalar.copy(out=xT_h[:], in_=ps_t[:])
                    xT.append(xT_h)

                # GEMM: out[t, o] = sum_k xT_k.T @ w_k  (fp32r, full rate)
                ps = acc_pool.tile([P, 2, 512], _F32)
                for k in range(KC):
                    lhsT = xT[k // 4][:, (k % 4) * P : (k % 4 + 1) * P]
                    for h in range(2):
                        nc.tensor.matmul(
                            ps[:, h, :],
                            lhsT,
                            w_sb[k][h][:],
                            start=(k == 0),
                            stop=(k == KC - 1),
                        )

                o_sb = opool.tile([P, D], _F32)
                for h in range(2):
                    nc.vector.tensor_tensor(
                        out=o_sb[:, h * 512 : (h + 1) * 512],
                        in0=ps[:, h, :],
                        in1=bias_sb[:, h * 512 : (h + 1) * 512],
                        op=mybir.AluOpType.add,
                    )
                nc.sync.dma_start(out_ext[i * P : (i + 1) * P, :], o_sb[:])

    if not nc.is_finalized():
        nc.finalize()
    return nc


def kernel(
    x,
    mem_fast,
    mem_medium,
    mem_slow,
    base_w,
    base_b,
    pd_w,
    pu_w,
    g1_w,
    g1_b,
    g2_w,
    g2_b,
):
    global LAST_RESULTS
    x = np.asarray(x, dtype=np.float32)
    mem = np.concatenate(
        [
            np.asarray(mem_fast, np.float32),
            np.asarray(mem_medium, np.float32),
            np.asarray(mem_slow, np.float32),
        ],
        axis=-1,
    )  # [B, 104]
    base_w = np.asarray(base_w, np.float32)
    base_b = np.asarray(base_b, np.float32)
    pd_w = np.asarray(pd_w, np.float32)
    pu_w = np.asarray(pu_w, np.float32)

    bias_bc = np.ascontiguousarray(
        np.broadcast_to(base_b[None, :], (P, D)), dtype=np.float32
    )

    in_maps = []
    for b in range(B):
        # Fold LoRA (and the constant 1/3 gate) into the base weight.
        scaled_pd = (1.0 + mem[b])[:, None].astype(np.float64) * pd_w.astype(
            np.float64
        )
        w_eff = base_w.astype(np.float64) + (ALPHA / 3.0) * (
            pu_w.astype(np.float64) @ scaled_pd
        )
        w_t = np.ascontiguousarray(w_eff.T, dtype=np.float32)  # [D_in, D_out]
        in_maps.append({"x": x[b], "w_t": w_t, "bias_bc": bias_bc})

    if "nc" not in _NC_CACHE:
        _NC_CACHE["nc"] = _build_nc()
    nc = _NC_CACHE["nc"]

    res = run_bass_kernel_spmd(nc, in_maps, list(range(B)))
    LAST_RESULTS = res
    out = np.stack([res.results[b]["out"] for b in range(B)], axis=0)
    return out.astype(np.float32)
